# revision 12
# baseline (speedup 1.0000x reference)
"""Trainium2 Bass kernel for nn_AGITransformer140B (8-core tensor-parallel).

Transformer block: h = x + Attn(RMSNorm(x)); out = h + SwiGLU(RMSNorm(h)).

Key simplification: the reference's second attention pass uses
rotate_half(Q), rotate_half(K) - which preserves both Q.K and Q^2.K^2 inner
products exactly, so out2 == out1 and the sigmoid gate is a no-op.  Only one
attention pass is computed.

Sharding: TP-8 over heads (2/core) and d_ff (1024/core).

v4 changes vs v3 baseline:
  * Emission order: ALL attention work (attn2/3, wo2/3) is emitted before
    anything that waits on the first AllReduce, so the PE never idles behind
    the collective (v3 had a 64us whole-chip stall there).
  * FFN output is produced FEATURE-major (y^T = w2^T @ act); the h/8 residual
    folds in feature-major directly from the AllReduce result, so the
    token-major transpose pass (256 PE transposes + 37us of scalar copies)
    is gone.  The final ReduceScatter scatters feature rows; the host
    transposes once at the end.
  * norm rstd via Ln+Exp (attention phase, shares the exp table set with
    softmax -> no table swaps) and Rsqrt (FFN phase, one op per chunk).
  * softmax 1/rowsum via reciprocal_approx_fast (~5x faster than DVE
    reciprocal).
  * scores evacuated through [128,1024] 2-bank PSUM tiles -> half the EXP
    instructions; rope runs at full-S width; QKV matmuls pair token-halves
    so each stationary weight load serves two matmuls.
  * tiny primer AllReduce at t=0 absorbs the first-collective overhead.
  * last FFN chunk's ReduceScatter is quartered by feature rows (contiguous
    slices) so the tail exposure is ~1/4 of a full RS.

Layouts (per core):
  xT       [D=2048, T=2048]  bf16 feature-major input (host-transposed)
  wqT/wkT  [2048, 256]  bf16 (norm1_w folded, head-dim pi-permuted: evens|odds)
  wvT      [2048, 256]  bf16 (norm1_w folded)
  woT      [256, 2048]  bf16
  w13T     [8, 2048, 256] bf16 df-major (norm2_w folded)
  w2T      [1024, 2048] bf16
  ropeC*/S* [64, 1024]  bf16 rope tables (Q tables pre-scaled by 1/sqrt(hd))
Output: out [256, T] bf16 feature-major; chunks 0-2: core c owns feature rows
c*256+j (cols k*512..); chunk 3 quarters: rows f*512+c*64+j at out rows
f*64+j, cols 1536..2048.
"""

import os
import sys
import types

sys.path.insert(0, "/opt/trn_rl_repo")

# ---- NTFF profile hook (boot() skips it: antenv stub lacks axon_hooks) ----
if "antenv.axon_hooks" not in sys.modules:
    _hooks_mod = types.ModuleType("antenv.axon_hooks")
    _HOOK = [None]
    _hooks_mod.set_axon_ntff_profile_hook = lambda h: _HOOK.__setitem__(0, h)
    _hooks_mod.get_axon_ntff_profile_hook = lambda: _HOOK[0]
    sys.modules["antenv.axon_hooks"] = _hooks_mod
    try:
        from trn_agent_boot.trn_boot import _ntff_profile_via_ctypes

        _hooks_mod.set_axon_ntff_profile_hook(
            _ntff_profile_via_ctypes("/opt/axon/libaxon_pjrt.so")
        )
    except Exception:
        pass

import ml_dtypes
import numpy as np

import concourse.bass as bass
import concourse.mybir as mybir
import concourse.tile as tile
from concourse import bacc
from concourse.bass_utils import run_bass_kernel_spmd
from concourse.masks import make_identity

BF16 = ml_dtypes.bfloat16
F32 = mybir.dt.float32
BF = mybir.dt.bfloat16
AF = mybir.ActivationFunctionType
ALU = mybir.AluOpType

N_CORES = 8
B, S, D, NH, HD, DFF = 2, 1024, 2048, 16, 128, 8192
T = B * S                      # 2048 tokens
NHL = NH // N_CORES            # 2 heads per core
DQ = NHL * HD                  # 256
DFL = DFF // N_CORES           # 1024
NDF = DFL // 128               # 8 dff tiles per core
NK = 4                         # token chunks for collective pipelining
CHUNK = T // NK                # 512
OUT_F = D // N_CORES           # 256 output feature rows per core
DT_TILES = D // 128            # 16
EPS = 1e-6
LAM = 0.1
HAD = 0.05
SQ_SCALE = float(np.sqrt(LAM * np.sqrt(HD)))   # fold lam*sqrt(hd) into Q^2


def build_nc():
    nc = bacc.Bacc("TRN2", target_bir_lowering=False, debug=False)

    xT_e = nc.declare_dram_parameter("xT", [B, DT_TILES, 128, S], BF,
                                     isOutput=False)
    wqT_e = nc.declare_dram_parameter("wqT", [D, DQ], BF, isOutput=False)
    wkT_e = nc.declare_dram_parameter("wkT", [D, DQ], BF, isOutput=False)
    wvT_e = nc.declare_dram_parameter("wvT", [D, DQ], BF, isOutput=False)
    woT_e = nc.declare_dram_parameter("woT", [DQ, D], BF, isOutput=False)
    w13T_e = nc.declare_dram_parameter("w13T", [NDF, 128, DT_TILES, 2 * 128],
                                       BF, isOutput=False)
    w2T_e = nc.declare_dram_parameter("w2T", [DFL, D], BF, isOutput=False)
    rCq_e = nc.declare_dram_parameter("ropeCq", [64, S], BF, isOutput=False)
    rSq_e = nc.declare_dram_parameter("ropeSq", [64, S], BF, isOutput=False)
    rCk_e = nc.declare_dram_parameter("ropeCk", [64, S], BF, isOutput=False)
    rSk_e = nc.declare_dram_parameter("ropeSk", [64, S], BF, isOutput=False)
    out_e = nc.declare_dram_parameter("out", [OUT_F, T], BF, isOutput=True)

    RG = [list(range(N_CORES))]

    with tile.TileContext(nc) as tc:
        with tc.tile_pool(name="const", bufs=1) as const, \
             tc.tile_pool(name="dram", bufs=1, space="DRAM") as dram:
            ident = const.tile([128, 128], BF)
            make_identity(nc, ident)
            ones128 = const.tile([128, 128], BF)
            nc.vector.memset(ones128[:], 1.0)
            eps_t = const.tile([128, 1], F32)
            nc.vector.memset(eps_t[:], EPS)

            # ---- PE warm-up: dense matmul chain bridges the DMA-load head
            # so HAM reaches K=8/8 before real compute starts ----
            warm_mv = const.tile([128, 256], BF)
            nc.vector.memset(warm_mv[:], 0.01)
            warm_sink = const.tile([128, 256], F32)
            ps_wu = None

            # ---- primer collective: absorb first-CC-call overhead ----
            prim_s = const.tile([128, 16], BF)
            nc.vector.memset(prim_s[:], 0.0)
            prim_i = dram.tile([128, 16], BF, tag="prim_i", name="prim_i")
            prim_o = dram.tile([128, 16], BF, tag="prim_o", name="prim_o")
            nc.sync.dma_start(prim_i[:], prim_s[:])
            nc.gpsimd.collective_compute(
                "AllReduce", ALU.add, replica_groups=RG,
                ins=[prim_i.opt()], outs=[prim_o.opt()],
            )

            rCq = const.tile([64, S], BF)
            rSq = const.tile([64, S], BF)
            rCk = const.tile([64, S], BF)
            rSk = const.tile([64, S], BF)

            RB1 = const.tile([128, T], BF)      # rstd bcast over partitions
            RSTM = const.tile([128, 16], F32)   # rstd token-major per tile

            ar_outs = [None] * NK

            # ---------- attention pools ----------
            # (warm-up chain uses ps_big; emitted after pool creation below)
            aops = tc.alloc_tile_pool(name="aops", bufs=1)
            wo_w = tc.alloc_tile_pool(name="wo_w", bufs=1)
            et_pool = tc.alloc_tile_pool(name="et", bufs=2)
            sm_pool = tc.alloc_tile_pool(name="sm", bufs=3)
            wo_ev = tc.alloc_tile_pool(name="wo_ev", bufs=4)
            ps_mm = tc.alloc_tile_pool(name="ps_mm", bufs=2, space="PSUM")
            ps_big = tc.alloc_tile_pool(name="ps_big", bufs=3, space="PSUM")

            WOT = wo_w.tile([128, NHL, D], BF, tag="wot")

            ps_wu = ps_big.tile([128, 256], F32, tag="ps_b", name="warm")
            for i in range(48):
                nc.tensor.matmul(ps_wu[:], ones128[:], warm_mv[:],
                                 start=(i == 0), stop=(i == 47))
            nc.scalar.copy(warm_sink[:], ps_wu[:])
            warm_dump = dram.tile([128, 256], F32, tag="warm_d", name="warm_d")
            nc.sync.dma_start(warm_dump[:], warm_sink[:])

            # ---------- batch-scoped pools ----------
            xnt1_pool = tc.alloc_tile_pool(name="xnt1_pool", bufs=1)
            wqkv = tc.alloc_tile_pool(name="wqkv", bufs=1)
            n1 = tc.alloc_tile_pool(name="n1", bufs=2)
            qkv_raw = tc.alloc_tile_pool(name="qkv_raw", bufs=2)
            rtmp = tc.alloc_tile_pool(name="rtmp", bufs=2)
            xnt0_pool = tc.alloc_tile_pool(name="xnt0_pool", bufs=1)

            XNT0 = xnt0_pool.tile([128, DT_TILES, S], BF, tag="xnt0")
            XNT1 = xnt1_pool.tile([128, DT_TILES, S], BF, tag="xnt1")
            # priority order: XNT0 + Q/K weights first (first compute),
            # then V/rope/WOT, then XNT1.  gpsimd issues NO dmas: its FIFO
            # holds only collective triggers.
            for dt in range(DT_TILES):
                eng = (nc.sync, nc.scalar)[dt % 2]
                eng.dma_start(XNT0[:, dt, :], xT_e[0, dt])
            WQ = wqkv.tile([128, DT_TILES, DQ], BF, tag="wq")
            WK = wqkv.tile([128, DT_TILES, DQ], BF, tag="wk")
            WV = wqkv.tile([128, DT_TILES, DQ], BF, tag="wv")
            nc.sync.dma_start(
                WQ[:], wqT_e.ap().rearrange("(dt p) m -> p dt m", p=128))
            nc.scalar.dma_start(
                WK[:], wkT_e.ap().rearrange("(dt p) m -> p dt m", p=128))
            nc.sync.dma_start(
                WV[:], wvT_e.ap().rearrange("(dt p) m -> p dt m", p=128))
            nc.scalar.dma_start(rCq[:], rCq_e[:])
            nc.sync.dma_start(rSq[:], rSq_e[:])
            nc.scalar.dma_start(rCk[:], rCk_e[:])
            nc.sync.dma_start(rSk[:], rSk_e[:])
            nc.scalar.dma_start(
                WOT[:], woT_e.ap().rearrange("(hm p) n -> p hm n", p=128))
            for dt in range(DT_TILES):
                eng = (nc.sync, nc.scalar)[dt % 2]
                eng.dma_start(XNT1[:, dt, :], xT_e[1, dt])

            def emit_norm1(b, XH):
                """RMSNorm1 rstd for batch b -> RB1[:, b*S:(b+1)*S], RSTM.

                rstd = exp(-0.5*ln(ms + eps)) keeps everything inside the
                natural_log_exp table set shared with softmax exp.
                """
                ps = ps_big.tile([128, 1024], F32, tag="ps_b", name=f"ssq{b}")
                for dt in range(DT_TILES):
                    sq = n1.tile([128, S], BF, tag="sq", name="sq")
                    if b == 0:
                        nc.vector.tensor_mul(sq[:], XH[:, dt, :], XH[:, dt, :])
                    else:
                        nc.scalar.activation(sq[:], XH[:, dt, :], AF.Square)
                    for nn in range(2):
                        nc.tensor.matmul(
                            ps[:, nn * 512:(nn + 1) * 512], ones128[:],
                            sq[:, nn * 512:(nn + 1) * 512],
                            start=(dt == 0), stop=(dt == DT_TILES - 1),
                        )
                cs = slice(b * S, (b + 1) * S)
                lnt = n1.tile([128, 1024], F32, tag="lnt", name="lnt")
                nc.scalar.activation(
                    lnt[:], ps[:], AF.Ln, scale=1.0 / D, bias=eps_t[:])
                nc.scalar.activation(RB1[:, cs], lnt[:], AF.Exp, scale=-0.5)
                # token-major rstd per 128-token tile (for V scaling)
                for tt in range(8):
                    gt = b * 8 + tt
                    ps_t = ps_big.tile([128, 128], BF, tag="ps_b", name="rstm_t")
                    nc.tensor.matmul(
                        ps_t[:], RB1[:, gt * 128:(gt + 1) * 128], ident[:],
                        is_transpose=True, start=True, stop=True)
                    nc.scalar.copy(RSTM[:, gt:gt + 1], ps_t[:, 0:1])

            def emit_qkv(b, XNTb, cur):
                """Q/K projections + rope for batch b; fills cur[...] tiles."""
                cs = slice(b * S, (b + 1) * S)
                cur["qtr"] = aops.tile([128, NHL, S], BF, tag="qtr",
                                       name=f"qtr{b}")
                cur["ktr"] = aops.tile([128, NHL, S], BF, tag="ktr",
                                       name=f"ktr{b}")
                cur["qsq"] = aops.tile([128, NHL, S], BF, tag="qsq",
                                       name=f"qsq{b}")
                cur["ksq"] = aops.tile([128, NHL, S], BF, tag="ksq",
                                       name=f"ksq{b}")
                cur["v"] = aops.tile([128, 8, DQ], BF, tag="v", name=f"v{b}")
                cur["ot"] = aops.tile([128, NHL, S], BF, tag="ot",
                                      name=f"ot{b}")
                for (W, rot_k, sq_k, CC, SS, sc_sq) in (
                    (WQ, "qtr", "qsq", rCq, rSq, SQ_SCALE),
                    (WK, "ktr", "ksq", rCk, rSk, 1.0),
                ):
                    ROT, SQT = cur[rot_k], cur[sq_k]
                    RAW = qkv_raw.tile([128, NHL, S], BF, tag="raw", name="raw")
                    for hm in range(NHL):
                        # token-half pair shares each stationary W slice
                        ps0 = ps_mm.tile([128, 512], F32, tag="ps_m",
                                         name="qk_ps0")
                        ps1 = ps_mm.tile([128, 512], F32, tag="ps_m",
                                         name="qk_ps1")
                        for dt in range(DT_TILES):
                            wsl = W[:, dt, hm * 128:(hm + 1) * 128]
                            nc.tensor.matmul(
                                ps0[:], wsl, XNTb[:, dt, 0:512],
                                start=(dt == 0), stop=(dt == DT_TILES - 1))
                            nc.tensor.matmul(
                                ps1[:], wsl, XNTb[:, dt, 512:1024],
                                start=(dt == 0), stop=(dt == DT_TILES - 1))
                        nc.scalar.copy(RAW[:, hm, 0:512], ps0[:])
                        nc.scalar.copy(RAW[:, hm, 512:1024], ps1[:])
                    # RoPE (pi layout: rows 0:64 = evens, 64:128 = odds)
                    for h in range(NHL):
                        x1 = RAW[0:64, h, :]
                        x2c = rtmp.tile([64, S], BF, tag="x2c", name="x2c")
                        nc.vector.tensor_copy(x2c[:], RAW[64:128, h, :])
                        tA = rtmp.tile([64, S], BF, tag="ta", name="ta")
                        tB = rtmp.tile([64, S], BF, tag="tb", name="tb")
                        nc.vector.tensor_mul(tA[:], x1, CC[:])
                        nc.vector.tensor_mul(tB[:], x2c[:], SS[:])
                        nc.vector.tensor_sub(ROT[0:64, h, :], tA[:], tB[:])
                        tC = rtmp.tile([64, S], BF, tag="ta", name="tc")
                        tD = rtmp.tile([64, S], BF, tag="tb", name="td")
                        nc.vector.tensor_mul(tC[:], x1, SS[:])
                        nc.vector.tensor_mul(tD[:], x2c[:], CC[:])
                        nc.vector.tensor_add(ROT[64:128, h, :], tC[:], tD[:])
                        nc.vector.tensor_mul(
                            ROT[:, h, :], ROT[:, h, :], RB1[:, cs])
                        nc.scalar.activation(
                            SQT[:, h, :], ROT[:, h, :], AF.Square, scale=sc_sq)

            def emit_v(b, XNTb, cur):
                Vt = cur["v"]
                for tt in range(8):
                    ps = ps_mm.tile([128, DQ], F32, tag="ps_m", name="v_ps")
                    for dt in range(DT_TILES):
                        nc.tensor.matmul(
                            ps[:],
                            XNTb[:, dt, tt * 128:(tt + 1) * 128],
                            WV[:, dt, :],
                            start=(dt == 0), stop=(dt == DT_TILES - 1),
                        )
                    gt = b * 8 + tt
                    nc.scalar.mul(Vt[:, tt, :], ps[:], RSTM[:, gt:gt + 1])

            def attn_chunk(k, cur):
                """scores+softmax+PV for 512-token chunk k (both heads)."""
                b, sc = divmod(k, 2)
                s0 = sc * 512       # batch-local
                QTr, KTr = cur["qtr"], cur["ktr"]
                Qsq, Ksq, Vt, OTb = cur["qsq"], cur["ksq"], cur["v"], cur["ot"]
                for h in range(NHL):
                    ET = et_pool.tile([128, 8, 512], BF, tag="et", name="et")
                    for tt2 in range(4):
                        ps_s = ps_big.tile([128, 1024], F32, tag="ps_b",
                                           name="ps_s")
                        for sub in range(2):
                            t0 = (tt2 * 2 + sub) * 128
                            o = sub * 512
                            nc.tensor.matmul(
                                ps_s[:, o:o + 512], KTr[:, h, t0:t0 + 128],
                                QTr[:, h, s0:s0 + 512],
                                start=True, stop=False,
                            )
                            nc.tensor.matmul(
                                ps_s[:, o:o + 512], Ksq[:, h, t0:t0 + 128],
                                Qsq[:, h, s0:s0 + 512],
                                start=False, stop=True,
                            )
                        nc.scalar.activation(
                            ET[:, tt2 * 2:tt2 * 2 + 2, :], ps_s[:], AF.Exp)
                    ps_cv = ps_big.tile([128, 1024], F32, tag="ps_b",
                                        name="ps_cv")
                    ps_c = ps_cv[:, 0:512]
                    ps_o = ps_cv[:, 512:1024]
                    for tt in range(8):
                        nc.tensor.matmul(
                            ps_c, ones128[:], ET[:, tt, :],
                            start=(tt == 0), stop=(tt == 7),
                        )
                        nc.tensor.matmul(
                            ps_o,
                            Vt[:, tt, h * 128:(h + 1) * 128],
                            ET[:, tt, :],
                            start=(tt == 0), stop=(tt == 7),
                        )
                    rc = sm_pool.tile([128, 512], F32, tag="rc", name="rc")
                    nc.vector.reciprocal_approx_fast(rc[:], ps_c)
                    t1 = sm_pool.tile([128, 512], F32, tag="t1", name="t1")
                    u1 = sm_pool.tile([128, 512], BF, tag="u1", name="u1")
                    nc.vector.tensor_mul(t1[:], ps_o, rc[:])
                    # out = t1 + HAD*t1^2 = t1 * (1 + HAD*t1)
                    nc.vector.tensor_scalar(
                        u1[:], t1[:], HAD, 1.0, ALU.mult, ALU.add)
                    nc.vector.tensor_mul(OTb[:, h, s0:s0 + 512], t1[:], u1[:])

            def emit_wo(k, XNTb, cur):
                """Feature-major wo projection + 0.125*x fold + AllReduce."""
                b, sc = divmod(k, 2)
                s0 = sc * 512
                OTb = cur["ot"]
                bounce = dram.tile([DT_TILES, 128, CHUNK], BF,
                                   tag=f"attn_b{k}", name=f"attn_b{k}")
                ars = []
                for dcol in range(DT_TILES):
                    ps_w = ps_mm.tile([128, 512], F32, tag="ps_m", name="ps_w")
                    for hm in range(NHL):
                        nc.tensor.matmul(
                            ps_w[:],
                            WOT[:, hm, dcol * 128:(dcol + 1) * 128],
                            OTb[:, hm, s0:s0 + 512],
                            start=(hm == 0), stop=(hm == NHL - 1),
                        )
                    st = wo_ev.tile([128, 512], BF, tag="st", name="st")
                    nc.vector.scalar_tensor_tensor(
                        st[:], XNTb[:, dcol, s0:s0 + 512], 0.125, ps_w[:],
                        ALU.mult, ALU.add,
                    )
                    (nc.sync, nc.scalar)[dcol % 2].dma_start(bounce[dcol], st[:])
                    if dcol % 8 == 7:
                        half = dcol // 8
                        ar_h = dram.tile([8, 128, CHUNK], BF,
                                         tag=f"ar{k}_{half}",
                                         name=f"ar{k}_{half}")
                        nc.gpsimd.collective_compute(
                            "AllReduce", ALU.add, replica_groups=RG,
                            ins=[bounce[half * 8:(half + 1) * 8].opt()],
                            outs=[ar_h.opt()],
                        )
                        ars.append(ar_h)
                ar_outs[k] = ars

            # ---------- attention emission: everything before any AR wait ----
            cur = {}
            with nc.named_scope("norm1a"):
                emit_norm1(0, XNT0)
            with nc.named_scope("qkvA"):
                emit_qkv(0, XNT0, cur)
                emit_v(0, XNT0, cur)
            with nc.named_scope("attn0"):
                attn_chunk(0, cur)
            with nc.named_scope("wo0"):
                emit_wo(0, XNT0, cur)
            with nc.named_scope("attn1"):
                attn_chunk(1, cur)
            with nc.named_scope("wo1"):
                emit_wo(1, XNT0, cur)
            xnt0_pool.release()
            with nc.named_scope("norm1b"):
                emit_norm1(1, XNT1)
            with nc.named_scope("qkvB"):
                emit_qkv(1, XNT1, cur)
                emit_v(1, XNT1, cur)
            rtmp.release()
            qkv_raw.release()
            n1.release()
            wqkv.release()
            with nc.named_scope("attn2"):
                attn_chunk(2, cur)
            with nc.named_scope("attn3"):
                attn_chunk(3, cur)
            with nc.named_scope("wo2"):
                emit_wo(2, XNT1, cur)
            with nc.named_scope("wo3"):
                emit_wo(3, XNT1, cur)

            xnt1_pool.release()
            for p in (ps_big, ps_mm, wo_ev, sm_pool, et_pool,
                      wo_w, aops):
                p.release()

            # ---------- FFN-persistent pools (right side) ----------
            ha_pool = tc.alloc_tile_pool(name="ha", bufs=2, side="right")
            xn2_pool = tc.alloc_tile_pool(name="xn2", bufs=2, side="right")
            n2w = tc.alloc_tile_pool(name="n2w", bufs=2, side="right")
            w2_pool = tc.alloc_tile_pool(name="w2r", bufs=1, side="right")
            ps_n2 = tc.alloc_tile_pool(name="ps_n2", bufs=1, space="PSUM",
                                       side="right")
            W2 = w2_pool.tile([128, NDF, D], BF, tag="w2r")
            nc.scalar.dma_start(
                W2[:], w2T_e.ap().rearrange("(df p) n -> p df n", p=128))

            def emit_ha(k):
                HA = ha_pool.tile([128, DT_TILES, CHUNK], BF, tag="ha",
                                  name=f"ha{k}")
                for half, eng in ((0, nc.sync), (1, nc.scalar)):
                    nc_src = ar_outs[k][half].rearrange("dt p t -> p dt t")
                    eng.dma_start(HA[:, half * 8:(half + 1) * 8, :], nc_src)
                return HA

            def emit_norm2(k):
                """h chunk from AllReduce -> xn2 (normed, feature-major)."""
                HA = emit_ha(k)
                ps_n = ps_n2.tile([128, 512], F32, tag="ps_n", name="ps_n")
                for dt in range(DT_TILES):
                    sq2 = n2w.tile([128, 512], BF, tag="sq2", name="sq2")
                    nc.scalar.activation(sq2[:], HA[:, dt, :], AF.Square)
                    nc.tensor.matmul(
                        ps_n[:], ones128[:], sq2[:],
                        start=(dt == 0), stop=(dt == DT_TILES - 1),
                    )
                ln2 = n2w.tile([128, 512], F32, tag="ln2", name="ln2")
                nc.scalar.activation(
                    ln2[:], ps_n[:], AF.Ln, scale=1.0 / D, bias=eps_t[:])
                r2b = n2w.tile([128, 512], BF, tag="r2b", name="r2b")
                nc.scalar.activation(r2b[:], ln2[:], AF.Exp, scale=-0.5)
                XN2 = xn2_pool.tile([128, DT_TILES, CHUNK], BF, tag="xn2",
                                    name=f"xn2{k}")
                for dt in range(DT_TILES):
                    nc.vector.tensor_mul(XN2[:, dt, :], HA[:, dt, :], r2b[:])
                return HA, XN2

            # ---------- FFN pools (left side) ----------
            ffn_w13 = tc.alloc_tile_pool(name="ffn_w13", bufs=4)
            ffn_act = tc.alloc_tile_pool(name="ffn_act", bufs=2)

            def w13_load(df):
                W13df = ffn_w13.tile([128, DT_TILES, 256], BF,
                                     tag="w13df", name="w13df")
                (nc.sync, nc.scalar)[df % 2].dma_start(W13df[:], w13T_e[df])
                return W13df

            ffn_ev = tc.alloc_tile_pool(name="ffn_ev", bufs=3)
            ffn_yo = tc.alloc_tile_pool(name="ffn_yo", bufs=2)
            ps_g = tc.alloc_tile_pool(name="ps_g", bufs=2, space="PSUM")
            ps_u = tc.alloc_tile_pool(name="ps_u", bufs=2, space="PSUM")
            ps_y = tc.alloc_tile_pool(name="ps_y", bufs=2, space="PSUM")

            def emit_gu(k, XN2, pre):
                ACT_K = ffn_act.tile([128, NDF, CHUNK], BF, tag="actk",
                                     name="actk")
                for df in range(NDF):
                    W13df = pre[df] if df < len(pre) else w13_load(df)
                    psg = ps_g.tile([128, 512], F32, tag="psg", name="psg")
                    psu = ps_u.tile([128, 512], F32, tag="psu", name="psu")
                    for dt in range(DT_TILES):
                        nc.tensor.matmul(
                            psg[:], W13df[:, dt, 0:128], XN2[:, dt, :],
                            start=(dt == 0), stop=(dt == DT_TILES - 1),
                        )
                    for dt in range(DT_TILES):
                        nc.tensor.matmul(
                            psu[:], W13df[:, dt, 128:256], XN2[:, dt, :],
                            start=(dt == 0), stop=(dt == DT_TILES - 1),
                        )
                    sg = ffn_ev.tile([128, 512], BF, tag="sg", name="sg")
                    nc.scalar.activation(sg[:], psg[:], AF.Silu)
                    nc.vector.tensor_mul(ACT_K[:, df, :], psu[:], sg[:])
                return ACT_K

            def emit_y(k, ACT_K, HA):
                """Feature-major y^T + h/8 fold + ReduceScatter over features."""
                bounce = dram.tile([D, CHUNK], BF, tag=f"ffn_b{k}",
                                   name=f"ffn_b{k}")
                last = (k == NK - 1)
                for dcol in range(DT_TILES):
                    psy = ps_y.tile([128, 512], F32, tag="psy", name="psy")
                    for df in range(NDF):
                        nc.tensor.matmul(
                            psy[:],
                            W2[:, df, dcol * 128:(dcol + 1) * 128],
                            ACT_K[:, df, :],
                            start=(df == 0), stop=(df == NDF - 1),
                        )
                    yo = ffn_yo.tile([128, CHUNK], BF, tag="yo", name="yo")
                    nc.vector.scalar_tensor_tensor(
                        yo[:], HA[:, dcol, :], 0.125, psy[:],
                        ALU.mult, ALU.add,
                    )
                    nc.sync.dma_start(
                        bounce[dcol * 128:(dcol + 1) * 128, :], yo[:])
                    if last and dcol % 4 == 3:
                        f = dcol // 4
                        rs_q = dram.tile([512 // N_CORES, CHUNK], BF,
                                         tag=f"ffn_rs{k}_{f}",
                                         name=f"ffn_rs{k}_{f}")
                        nc.gpsimd.collective_compute(
                            "ReduceScatter", ALU.add, replica_groups=RG,
                            ins=[bounce[f * 512:(f + 1) * 512, :].opt()],
                            outs=[rs_q.opt()],
                        )
                        nc.scalar.dma_start(
                            out_e[f * 64:(f + 1) * 64,
                                  k * CHUNK:(k + 1) * CHUNK],
                            rs_q[:])
                if not last:
                    ffn_rs = dram.tile([OUT_F, CHUNK], BF, tag=f"ffn_rs{k}",
                                       name=f"ffn_rs{k}")
                    nc.gpsimd.collective_compute(
                        "ReduceScatter", ALU.add, replica_groups=RG,
                        ins=[bounce.opt()], outs=[ffn_rs.opt()],
                    )
                    nc.scalar.dma_start(
                        out_e[:, k * CHUNK:(k + 1) * CHUNK], ffn_rs[:])

            norm2_out = [None] * NK
            with nc.named_scope("n2_0"):
                norm2_out[0] = emit_norm2(0)
            pre = [w13_load(df) for df in range(4)]
            for k in range(NK):
                with nc.named_scope(f"ffn{k}"):
                    HA, XN2 = norm2_out[k]
                    ACT_K = emit_gu(k, XN2, pre)
                    if k < NK - 1:
                        with nc.named_scope(f"n2_{k + 1}"):
                            norm2_out[k + 1] = emit_norm2(k + 1)
                        pre = [w13_load(df) for df in range(4)]
                    with nc.named_scope(f"y{k}"):
                        emit_y(k, ACT_K, HA)

            for p in (ps_y, ps_u, ps_g, ffn_yo, ffn_ev, ffn_act):
                p.release()
            ffn_w13.release()
            for p in (ps_n2, w2_pool, n2w, xn2_pool, ha_pool):
                p.release()

    nc.compile()
    return nc


_NC_CACHE = None


def _get_nc():
    global _NC_CACHE
    if _NC_CACHE is None:
        _NC_CACHE = build_nc()
    return _NC_CACHE


def prep_inputs(x, norm1_w, norm2_w, wq, wk, wv, wo, gate_w, w1, w3, w2):
    """Build the 8 per-core input maps (host-side sharding + layout prep)."""
    x2d = np.ascontiguousarray(np.asarray(x, np.float32).reshape(T, D))
    xT_full = x2d.T.astype(BF16)                      # [D, T]
    xT = np.ascontiguousarray(
        xT_full.reshape(DT_TILES, 128, B, S).transpose(2, 0, 1, 3))
    pi = np.concatenate([np.arange(0, HD, 2), np.arange(1, HD, 2)])
    inv = 1.0 / (10000.0 ** (np.arange(0, HD, 2, dtype=np.float64) / HD))
    ang = np.arange(S, dtype=np.float64)[:, None] * inv[None, :]   # [S, 64]
    Ct = np.ascontiguousarray(np.cos(ang).T).astype(np.float32)    # [64, S]
    St = np.ascontiguousarray(np.sin(ang).T).astype(np.float32)
    qs = 1.0 / np.sqrt(HD)
    rCq = (Ct * qs).astype(BF16)
    rSq = (St * qs).astype(BF16)
    rCk = Ct.astype(BF16)
    rSk = St.astype(BF16)

    n1 = np.asarray(norm1_w, np.float32)
    n2 = np.asarray(norm2_w, np.float32)
    wq = np.asarray(wq, np.float32)
    wk = np.asarray(wk, np.float32)
    wv = np.asarray(wv, np.float32)
    wo = np.asarray(wo, np.float32)
    w1 = np.asarray(w1, np.float32)
    w3 = np.asarray(w3, np.float32)
    w2 = np.asarray(w2, np.float32)

    in_maps = []
    for c in range(N_CORES):
        r0, r1 = c * DQ, (c + 1) * DQ
        wq_c = (wq[r0:r1, :] * n1[None, :]).reshape(NHL, HD, D)[:, pi, :].reshape(DQ, D)
        wk_c = (wk[r0:r1, :] * n1[None, :]).reshape(NHL, HD, D)[:, pi, :].reshape(DQ, D)
        wv_c = wv[r0:r1, :] * n1[None, :]
        f0, f1 = c * DFL, (c + 1) * DFL
        w1_c = (w1[f0:f1, :] * n2[None, :]).T.astype(BF16)   # [D, DFL]
        w3_c = (w3[f0:f1, :] * n2[None, :]).T.astype(BF16)
        # df-major interleaved: [NDF, D, 256] = [w1 | w3] per df-tile
        w1_df = w1_c.reshape(D, NDF, 128).transpose(1, 0, 2)
        w3_df = w3_c.reshape(D, NDF, 128).transpose(1, 0, 2)
        w13_df = np.concatenate([w1_df, w3_df], axis=2)   # [NDF, D, 256]
        # [NDF, 128, DT, 256]: 8KB contiguous per partition row per df-tile
        w13_df = np.ascontiguousarray(
            w13_df.reshape(NDF, DT_TILES, 128, 256).transpose(0, 2, 1, 3))
        in_maps.append({
            "xT": xT,
            "wqT": np.ascontiguousarray(wq_c.T).astype(BF16),
            "wkT": np.ascontiguousarray(wk_c.T).astype(BF16),
            "wvT": np.ascontiguousarray(wv_c.T).astype(BF16),
            "woT": np.ascontiguousarray(wo[:, r0:r1].T).astype(BF16),
            "w13T": w13_df,
            "w2T": np.ascontiguousarray(w2[:, f0:f1].T).astype(BF16),
            "ropeCq": rCq, "ropeSq": rSq, "ropeCk": rCk, "ropeSk": rSk,
        })
    return in_maps


def unshard_output(results):
    outT = np.empty((D, T), np.float32)
    for c in range(N_CORES):
        oc = np.asarray(results[c]["out"], dtype=np.float32)   # [256, T]
        # chunks 0..2: core c owns contiguous feature rows c*256..(c+1)*256
        outT[c * OUT_F:(c + 1) * OUT_F, 0:(NK - 1) * CHUNK] = \
            oc[:, 0:(NK - 1) * CHUNK]
        # chunk 3 quarters: quarter f scatters rows f*512+c*64..+64
        for f in range(4):
            g0 = f * 512 + c * 64
            outT[g0:g0 + 64, (NK - 1) * CHUNK:] = \
                oc[f * 64:(f + 1) * 64, (NK - 1) * CHUNK:]
    return np.ascontiguousarray(outT.T).reshape(B, S, D)


def run(in_maps, trace=False):
    nc = _get_nc()
    return run_bass_kernel_spmd(nc, in_maps, core_ids=list(range(N_CORES)), trace=trace)


def kernel(**inputs):
    in_maps = prep_inputs(**inputs)
    res = run(in_maps, trace=False)
    return unshard_output(res.results)
